# revision 17
# baseline (speedup 1.0000x reference)
"""DiffeomorphicTransform (scaling-and-squaring of a stationary velocity
field) on 8 Trainium2 NeuronCores via Bass/Tile.

Reference (per step, 7 steps):
    x = grid_w + flow[:,2]; y = grid_h + flow[:,1]; z = grid_d + flow[:,0]
    flow = flow + trilinear_sample_border(flow, x, y, z)

Distribution: cores 0-3 compute batch 0, cores 4-7 batch 1; within a
batch-group each core owns a 40-plane z-slab of the output.  After every
step the four cores of a group AllGather their slabs into the full-volume
buffer used to build the next step's gather table.

Gather strategy: flow is kept channel-last; each step it is expanded into a
4x-duplicated "corner cube" table TAB[v, (zi,yi,c)] = flow[v + zi*H*W +
yi*W, c].  All 8 trilinear corners x 3 channels of one output voxel are
then one contiguous 96 B run at row z0*H*W + y0*W + x0 -- a single fused
indirect-DMA (128*SN offsets, one 96 B descriptor per voxel) per chunk.

I/O: inputs are per-channel planar slabs (zero-copy host views of the
original [B,C,D,H,W] array); the 1/2^7 scaling and channel interleave run
on-device in the prologue.  Grid coordinates are uint8 voxel indices
(z global), expanded to f32 on-device.  The final step writes the output
planar so the host reassembles with contiguous copies only.
"""

import os as _os
import numpy as np

TIME_STEP = int(_os.environ.get("KSTEPS", "7"))
# one offset per partition is a HW invariant of indirect DMA (verified by
# probe): the per-j form is the only correct gather shape.
GATHER_MODE = _os.environ.get("KGATHER", "perj")  # perj | fused | plain
B, C = 2, 3
D = H = W = 160
PLANE = H * W
VOL = D * PLANE
NCORES = 8
GROUP = 4                        # cores per batch
SLABZ = D // GROUP
SLABVOX = SLABZ * PLANE
PADVOX = PLANE + W + 2           # expand reads up to v + PLANE + W + 1
VOLP = VOL + PADVOX
FSCALE = 1.0 / (2.0 ** 7)        # velocity scaling folded into prologue

SN = 320                         # sample-chunk voxels per partition
CVOX = 128 * SN
NCH = SLABVOX // CVOX            # 25
EM = 500                         # expand-chunk voxels per partition
EVOX = 128 * EM
NECH = VOL // EVOX               # 64

assert NCH * CVOX == SLABVOX and NECH * EVOX == VOL

_CACHE = {}


def _build_nc():
    import concourse.bass as bass
    import concourse.bacc as bacc
    import concourse.tile as tile
    import concourse.mybir as mybir

    F32 = mybir.dt.float32
    I32 = mybir.dt.int32
    U8 = mybir.dt.uint8
    AL = mybir.AluOpType
    ts = bass.ts

    nc = bacc.Bacc("TRN2", target_bir_lowering=False, debug=False,
                   enable_asserts=False, num_devices=NCORES)

    slabin = [nc.dram_tensor(f"slab{c}", [SLABVOX], F32,
                             kind="ExternalInput").ap() for c in range(3)]
    coordu = nc.dram_tensor("coordu", [3, SLABVOX], U8,
                            kind="ExternalInput").ap()
    fout = nc.dram_tensor("fout", [3, SLABVOX], F32, kind="ExternalOutput").ap()

    TAB = nc.dram_tensor("tabT", [VOL, 12], F32, kind="Internal").ap()
    FULL = nc.dram_tensor("fullT", [VOLP, 3], F32, kind="Internal").ap()
    SLAB = nc.dram_tensor("slabT", [SLABVOX, 3], F32, kind="Internal").ap()

    replica_groups = [list(range(GROUP)), list(range(GROUP, 2 * GROUP))]
    TAB_flat = TAB.rearrange("v q -> (v q)")

    def emit_allgather(tag, halves=1):
        """AllGather SLAB -> FULL, optionally as `halves` range-split
        collectives so the first can start while late sample chunks still
        run (if DRAM deps are range-tracked)."""
        n3 = SLABVOX * 3
        part = n3 // halves
        with nc.named_scope(tag):
            for h in range(halves):
                nc.gpsimd.collective_compute(
                    "AllGather", mybir.AluOpType.bypass,
                    replica_groups=replica_groups,
                    ins=[SLAB.rearrange("v c -> (v c)")
                         [h * part:(h + 1) * part]
                         .rearrange("(g n) -> g n", g=1)],
                    outs=[FULL.rearrange("v c -> (v c)")[0:VOL * 3]
                          .rearrange("(g n) -> g n", g=GROUP)
                          [:, h * part:(h + 1) * part]],
                )

    with tile.TileContext(nc) as tc:
        # ---- prologue: interleave planar channel slabs -> SLAB (x 1/128),
        # then AllGather SLAB -> FULL ----
        slf = SLAB.rearrange("v c -> (v c)")
        with nc.named_scope("pro"), tc.tile_pool(name="pro", bufs=2) as pp:
            def pbody(ci):
                f = pp.tile([128, SN * 3], F32, tag="f")
                f3 = f[:].rearrange("p (nv c) -> p nv c", c=3)
                for c in range(3):
                    ld = pp.tile([128, SN], F32, tag=f"ld{c}")
                    nc.sync.dma_start(
                        ld[:],
                        slabin[c][ts(ci, CVOX)]
                        .rearrange("(p x) -> p x", p=128))
                    nc.vector.tensor_scalar_mul(f3[:, :, c], ld[:], FSCALE)
                nc.sync.dma_start(
                    slf[ts(ci, CVOX * 3)].rearrange("(p x) -> p x", p=128),
                    f[:])
            with tc.For_i(0, NCH, 1) as ci:
                pbody(ci)
        emit_allgather("ag_pro")

        for step in range(TIME_STEP):
            last = step == TIME_STEP - 1
            src_full_flat = FULL.rearrange("v c -> (v c)")
            src_slab_flat = SLAB.rearrange("v c -> (v c)")

            # ---- phase B: expand FULL -> TAB (corner-cube table) ----
            with nc.named_scope(f"ep{step}"), \
                 tc.tile_pool(name=f"ep{step}", bufs=2) as ep:
                def ebody(ci):
                    tch = ep.tile([128, EM * 12], F32, tag="tch")
                    tch3 = tch[:].rearrange("p (m q) -> p m q", q=12)
                    # keep Pool free: it owns the 8000 gather descgen
                    # instructions per step (994 ns fixed each, serialized)
                    engs = [nc.vector, nc.vector, nc.vector, None]
                    for q, (zi, yi) in enumerate(((0, 0), (0, 1), (1, 0),
                                                  (1, 1))):
                        ld = ep.tile([128, EM * 3], F32, tag=f"ld{q}")
                        shift = (zi * PLANE + yi * W) * 3
                        nc.sync.dma_start(
                            ld[:],
                            src_full_flat[shift:shift + NECH * EVOX * 3]
                            [ts(ci, EVOX * 3)]
                            .rearrange("(p x) -> p x", p=128))
                        src3 = ld[:].rearrange("p (m c) -> p m c", c=3)
                        dst3 = tch3[:, :, q * 3:(q + 1) * 3]
                        if engs[q] is None:
                            nc.scalar.copy(dst3, src3)
                        else:
                            engs[q].tensor_copy(out=dst3, in_=src3)
                    nc.sync.dma_start(
                        TAB_flat[ts(ci, EVOX * 12)]
                        .rearrange("(p x) -> p x", p=128),
                        tch[:])
                with tc.For_i(0, NECH, 1) as ci:
                    ebody(ci)

            # ---- phase C: sample + add, write new slab / planar fout ----
            with nc.named_scope(f"sp{step}"), \
                 tc.tile_pool(name=f"sp{step}", bufs=2) as sp:
                def sbody(ci):
                    f = sp.tile([128, SN * 3], F32, tag="f")
                    nc.sync.dma_start(
                        f[:],
                        src_slab_flat[ts(ci, CVOX * 3)]
                        .rearrange("(p x) -> p x", p=128))
                    f3 = f[:].rearrange("p (nv c) -> p nv c", c=3)

                    cg = []
                    for dc in range(3):
                        tu = sp.tile([128, SN], U8, tag=f"cu{dc}")
                        nc.sync.dma_start(
                            tu[:],
                            coordu[dc, :][ts(ci, CVOX)]
                            .rearrange("(p x) -> p x", p=128))
                        t = sp.tile([128, SN], F32, tag=f"cg{dc}")
                        nc.scalar.copy(out=t[:], in_=tu[:])
                        cg.append(t)
                    zg, yg, xg = cg          # integer voxel coords as f32

                    ws = []
                    for comp, (g, hi) in enumerate(((zg, D), (yg, H),
                                                    (xg, W))):
                        # p = clip(coord + flow*scale, 0, hi-1)
                        scale = (hi - 1) / 2.0
                        p_ = sp.tile([128, SN], F32, tag=f"p{comp}")
                        nc.vector.scalar_tensor_tensor(
                            out=p_[:], in0=f3[:, :, comp],
                            scalar=float(scale), in1=g[:],
                            op0=AL.mult, op1=AL.add)
                        nc.vector.tensor_scalar_max(p_[:], p_[:], 0.0)
                        nc.vector.tensor_scalar_min(p_[:], p_[:],
                                                    float(hi - 1))
                        # f32->i32 copy rounds to nearest; round(p-0.5) gives
                        # floor(p) (or floor-1 with weight 1 -- same lerp)
                        ph = sp.tile([128, SN], F32, tag=f"ph{comp}")
                        nc.vector.tensor_scalar_add(ph[:], p_[:], -0.5)
                        i0 = sp.tile([128, SN], I32, tag=f"i{comp}")
                        nc.vector.tensor_copy(out=i0[:], in_=ph[:])
                        f0 = sp.tile([128, SN], F32, tag=f"ff{comp}")
                        nc.vector.tensor_copy(out=f0[:], in_=i0[:])
                        nc.vector.tensor_scalar_min(f0[:], f0[:],
                                                    float(hi - 2))
                        w = sp.tile([128, SN], F32, tag=f"w{comp}")
                        nc.vector.tensor_tensor(out=w[:], in0=p_[:],
                                                in1=f0[:], op=AL.subtract)
                        ws.append((f0, w))
                    (z0f, wz), (y0f, wy), (x0f, wx) = ws

                    idxf = sp.tile([128, SN], F32, tag="idxf")
                    nc.vector.scalar_tensor_tensor(
                        out=idxf[:], in0=z0f[:], scalar=float(H), in1=y0f[:],
                        op0=AL.mult, op1=AL.add)
                    nc.vector.scalar_tensor_tensor(
                        out=idxf[:], in0=idxf[:], scalar=float(W), in1=x0f[:],
                        op0=AL.mult, op1=AL.add)
                    idxi = sp.tile([128, SN], I32, tag="idxi")
                    nc.vector.tensor_copy(out=idxi[:], in_=idxf[:])

                    g24 = sp.tile([128, SN * 24], F32, tag="g24")
                    g243 = g24[:].rearrange("p (nv k) -> p nv k", k=24)
                    if GATHER_MODE == "fused":
                        # HW emits one descriptor per 12-elem TAB row, one
                        # offset per run: interleave (idx, idx+1) so each
                        # voxel's 24-float window = rows idx, idx+1.
                        idx2 = sp.tile([128, SN * 2], I32, tag="idx2")
                        idx22 = idx2[:].rearrange("p (nv k) -> p nv k", k=2)
                        nc.vector.tensor_copy(out=idx22[:, :, 0],
                                              in_=idxi[:])
                        nc.vector.tensor_scalar_add(idx22[:, :, 1],
                                                    idxi[:], 1)
                        nc.gpsimd.indirect_dma_start(
                            out=g24[:], out_offset=None, in_=TAB,
                            in_offset=bass.IndirectOffsetOnAxis(
                                ap=idx2[:], axis=0))
                    elif GATHER_MODE == "perj":
                        for j in range(SN):
                            nc.gpsimd.indirect_dma_start(
                                out=g243[:, j, :], out_offset=None, in_=TAB,
                                in_offset=bass.IndirectOffsetOnAxis(
                                    ap=idxi[:, j:j + 1], axis=0))
                    else:  # plain: timing-structure stub, wrong results
                        nc.sync.dma_start(
                            g24[:],
                            TAB_flat[0:CVOX * 24]
                            .rearrange("(p x) -> p x", p=128))

                    # x-lerp (pairs 12 apart) -> z-lerp (6) -> y-lerp (3)
                    t1 = sp.tile([128, SN * 12], F32, tag="t1")
                    t13 = t1[:].rearrange("p (nv k) -> p nv k", k=12)
                    a, b = g243[:, :, 0:12], g243[:, :, 12:24]
                    nc.vector.tensor_tensor(out=t13, in0=b, in1=a,
                                            op=AL.subtract)
                    nc.vector.tensor_tensor(
                        out=t13, in0=t13,
                        in1=wx[:].to_broadcast([128, SN, 12]), op=AL.mult)
                    nc.vector.tensor_tensor(out=t13, in0=t13, in1=a,
                                            op=AL.add)
                    t2 = sp.tile([128, SN * 6], F32, tag="t2")
                    t23 = t2[:].rearrange("p (nv k) -> p nv k", k=6)
                    a, b = t13[:, :, 0:6], t13[:, :, 6:12]
                    nc.vector.tensor_tensor(out=t23, in0=b, in1=a,
                                            op=AL.subtract)
                    nc.vector.tensor_tensor(
                        out=t23, in0=t23,
                        in1=wz[:].to_broadcast([128, SN, 6]), op=AL.mult)
                    nc.vector.tensor_tensor(out=t23, in0=t23, in1=a,
                                            op=AL.add)
                    t3 = sp.tile([128, SN * 3], F32, tag="t3")
                    t33 = t3[:].rearrange("p (nv k) -> p nv k", k=3)
                    a, b = t23[:, :, 0:3], t23[:, :, 3:6]
                    nc.vector.tensor_tensor(out=t33, in0=b, in1=a,
                                            op=AL.subtract)
                    nc.vector.tensor_tensor(
                        out=t33, in0=t33,
                        in1=wy[:].to_broadcast([128, SN, 3]), op=AL.mult)
                    nc.vector.tensor_tensor(out=t33, in0=t33, in1=a,
                                            op=AL.add)

                    if not last:
                        outt = sp.tile([128, SN * 3], F32, tag="outt")
                        nc.vector.tensor_tensor(out=outt[:], in0=t3[:],
                                                in1=f[:], op=AL.add)
                        nc.sync.dma_start(
                            src_slab_flat[ts(ci, CVOX * 3)]
                            .rearrange("(p x) -> p x", p=128),
                            outt[:])
                    else:
                        # planar output: out_c = t33[:,:,c] + f3[:,:,c]
                        for c in range(3):
                            oc = sp.tile([128, SN], F32, tag=f"oc{c}")
                            nc.vector.tensor_tensor(
                                out=oc[:], in0=t33[:, :, c],
                                in1=f3[:, :, c], op=AL.add)
                            nc.sync.dma_start(
                                fout[c, :][ts(ci, CVOX)]
                                .rearrange("(p x) -> p x", p=128),
                                oc[:])
                with tc.For_i(0, NCH, 1) as ci:
                    sbody(ci)

            # ---- phase D: AllGather new slabs into FULL ----
            # (halves=2 split is rejected by the BIR verifier: collective
            # outs must be contiguous, and half-gathers stride across FULL)
            if not last:
                emit_allgather(f"ag{step}")

    nc.compile()
    return nc


def _get_nc():
    if "nc" not in _CACHE:
        _CACHE["nc"] = _build_nc()
    return _CACHE["nc"]


def _make_coords_u8():
    """Per-slab uint8 integer voxel coordinates, rows (z_global, y, x)."""
    if "coords" in _CACHE:
        return _CACHE["coords"]
    zz = np.arange(D, dtype=np.uint8)
    yy = np.arange(H, dtype=np.uint8)
    xx = np.arange(W, dtype=np.uint8)
    out = []
    for s in range(GROUP):
        z_v, y_v, x_v = np.meshgrid(zz[s * SLABZ:(s + 1) * SLABZ], yy, xx,
                                    indexing="ij")
        c = np.stack([z_v.ravel(), y_v.ravel(), x_v.ravel()])
        out.append(np.ascontiguousarray(c))
    _CACHE["coords"] = out
    return out


def kernel(flow: np.ndarray) -> np.ndarray:
    from concourse.bass_utils import run_bass_kernel_spmd

    assert flow.shape == (B, C, D, H, W), flow.shape
    nc = _get_nc()
    flow = np.ascontiguousarray(flow, dtype=np.float32)
    coords = _make_coords_u8()

    in_maps = []
    for core in range(NCORES):
        b, s = divmod(core, GROUP)
        zsl = slice(s * SLABZ, (s + 1) * SLABZ)
        m = {f"slab{c}": flow[b, c, zsl].reshape(-1) for c in range(3)}
        m["coordu"] = coords[s]
        in_maps.append(m)

    res = run_bass_kernel_spmd(nc, in_maps, core_ids=list(range(NCORES)))

    out = np.empty((B, C, D, H, W), dtype=np.float32)
    for core in range(NCORES):
        b, s = divmod(core, GROUP)
        zsl = slice(s * SLABZ, (s + 1) * SLABZ)
        out[b, :, zsl] = res.results[core]["fout"].reshape(3, SLABZ, H, W)
    return out


# revision 18
# speedup vs baseline: 1.1514x; 1.1514x over previous
"""DiffeomorphicTransform (scaling-and-squaring of a stationary velocity
field) on 8 Trainium2 NeuronCores via Bass/Tile.

Reference (per step, 7 steps):
    x = grid_w + flow[:,2]; y = grid_h + flow[:,1]; z = grid_d + flow[:,0]
    flow = flow + trilinear_sample_border(flow, x, y, z)

Distribution: cores 0-3 compute batch 0, cores 4-7 batch 1; within a
batch-group each core owns a 40-plane z-slab of the output.  After every
step the four cores of a group AllGather their slabs into the full-volume
buffer used to build the next step's gather table.

Gather strategy: flow is kept channel-last; each step it is expanded into a
4x-duplicated "corner cube" table TAB[v, (zi,yi,c)] = flow[v + zi*H*W +
yi*W, c].  All 8 trilinear corners x 3 channels of one output voxel are
then one contiguous 96 B run at row z0*H*W + y0*W + x0 -- a single fused
indirect-DMA (128*SN offsets, one 96 B descriptor per voxel) per chunk.

I/O: inputs are per-channel planar slabs (zero-copy host views of the
original [B,C,D,H,W] array); the 1/2^7 scaling and channel interleave run
on-device in the prologue.  Grid coordinates are uint8 voxel indices
(z global), expanded to f32 on-device.  The final step writes the output
planar so the host reassembles with contiguous copies only.
"""

import os as _os
import numpy as np

TIME_STEP = int(_os.environ.get("KSTEPS", "7"))
# one offset per partition is a HW invariant of indirect DMA (verified by
# probe): the per-j form is the only correct gather shape.
GATHER_MODE = _os.environ.get("KGATHER", "perj")  # perj | fused | plain
B, C = 2, 3
D = H = W = 160
PLANE = H * W
VOL = D * PLANE
NCORES = 8
GROUP = 4                        # cores per batch
SLABZ = D // GROUP
SLABVOX = SLABZ * PLANE
PADVOX = PLANE + W + 2           # expand reads up to v + PLANE + W + 1
VOLP = VOL + PADVOX
FSCALE = 1.0 / (2.0 ** 7)        # velocity scaling folded into prologue

SN = 320                         # sample-chunk voxels per partition
CVOX = 128 * SN
NCH = SLABVOX // CVOX            # 25
EM = 500                         # expand-chunk voxels per partition
EVOX = 128 * EM
NECH = VOL // EVOX               # 64

assert NCH * CVOX == SLABVOX and NECH * EVOX == VOL

_CACHE = {}


def _build_nc():
    import concourse.bass as bass
    import concourse.bacc as bacc
    import concourse.tile as tile
    import concourse.mybir as mybir

    F32 = mybir.dt.float32
    I32 = mybir.dt.int32
    U8 = mybir.dt.uint8
    AL = mybir.AluOpType
    ts = bass.ts

    nc = bacc.Bacc("TRN2", target_bir_lowering=False, debug=False,
                   enable_asserts=False, num_devices=NCORES)

    slabin = [nc.dram_tensor(f"slab{c}", [SLABVOX], F32,
                             kind="ExternalInput").ap() for c in range(3)]
    coordu = nc.dram_tensor("coordu", [3, SLABVOX], U8,
                            kind="ExternalInput").ap()
    fout = nc.dram_tensor("fout", [3, SLABVOX], F32, kind="ExternalOutput").ap()

    TAB = nc.dram_tensor("tabT", [VOL, 12], F32, kind="Internal").ap()
    FULL = nc.dram_tensor("fullT", [VOLP, 3], F32, kind="Internal").ap()
    SLAB = nc.dram_tensor("slabT", [SLABVOX, 3], F32, kind="Internal").ap()

    replica_groups = [list(range(GROUP)), list(range(GROUP, 2 * GROUP))]
    TAB_flat = TAB.rearrange("v q -> (v q)")

    def emit_allgather(tag, halves=1):
        """AllGather SLAB -> FULL, optionally as `halves` range-split
        collectives so the first can start while late sample chunks still
        run (if DRAM deps are range-tracked)."""
        n3 = SLABVOX * 3
        part = n3 // halves
        with nc.named_scope(tag):
            for h in range(halves):
                nc.gpsimd.collective_compute(
                    "AllGather", mybir.AluOpType.bypass,
                    replica_groups=replica_groups,
                    ins=[SLAB.rearrange("v c -> (v c)")
                         [h * part:(h + 1) * part]
                         .rearrange("(g n) -> g n", g=1)],
                    outs=[FULL.rearrange("v c -> (v c)")[0:VOL * 3]
                          .rearrange("(g n) -> g n", g=GROUP)
                          [:, h * part:(h + 1) * part]],
                )

    with tile.TileContext(nc) as tc:
        # ---- prologue: interleave planar channel slabs -> SLAB (x 1/128),
        # then AllGather SLAB -> FULL ----
        slf = SLAB.rearrange("v c -> (v c)")
        with nc.named_scope("pro"), tc.tile_pool(name="pro", bufs=2) as pp:
            def pbody(ci):
                f = pp.tile([128, SN * 3], F32, tag="f")
                f3 = f[:].rearrange("p (nv c) -> p nv c", c=3)
                for c in range(3):
                    ld = pp.tile([128, SN], F32, tag=f"ld{c}")
                    nc.sync.dma_start(
                        ld[:],
                        slabin[c][ts(ci, CVOX)]
                        .rearrange("(p x) -> p x", p=128))
                    nc.vector.tensor_scalar_mul(f3[:, :, c], ld[:], FSCALE)
                nc.sync.dma_start(
                    slf[ts(ci, CVOX * 3)].rearrange("(p x) -> p x", p=128),
                    f[:])
            with tc.For_i(0, NCH, 1) as ci:
                pbody(ci)
        emit_allgather("ag_pro")

        for step in range(TIME_STEP):
            last = step == TIME_STEP - 1
            src_full_flat = FULL.rearrange("v c -> (v c)")
            src_slab_flat = SLAB.rearrange("v c -> (v c)")

            # ---- phase B: expand FULL -> TAB (corner-cube table) ----
            with nc.named_scope(f"ep{step}"), \
                 tc.tile_pool(name=f"ep{step}", bufs=2) as ep:
                def ebody(ci):
                    tch = ep.tile([128, EM * 12], F32, tag="tch")
                    tch3 = tch[:].rearrange("p (m q) -> p m q", q=12)
                    # keep Pool free: it owns the 8000 gather descgen
                    # instructions per step (994 ns fixed each, serialized)
                    engs = [nc.vector, nc.vector, nc.vector, None]
                    for q, (zi, yi) in enumerate(((0, 0), (0, 1), (1, 0),
                                                  (1, 1))):
                        ld = ep.tile([128, EM * 3], F32, tag=f"ld{q}")
                        shift = (zi * PLANE + yi * W) * 3
                        nc.sync.dma_start(
                            ld[:],
                            src_full_flat[shift:shift + NECH * EVOX * 3]
                            [ts(ci, EVOX * 3)]
                            .rearrange("(p x) -> p x", p=128))
                        src3 = ld[:].rearrange("p (m c) -> p m c", c=3)
                        dst3 = tch3[:, :, q * 3:(q + 1) * 3]
                        if engs[q] is None:
                            nc.scalar.copy(dst3, src3)
                        else:
                            engs[q].tensor_copy(out=dst3, in_=src3)
                    nc.sync.dma_start(
                        TAB_flat[ts(ci, EVOX * 12)]
                        .rearrange("(p x) -> p x", p=128),
                        tch[:])
                with tc.For_i(0, NECH, 1) as ci:
                    ebody(ci)

            # ---- phase C: sample + add, write new slab / planar fout ----
            with nc.named_scope(f"sp{step}"), \
                 tc.tile_pool(name=f"sp{step}", bufs=2) as sp:
                def sbody(ci):
                    f = sp.tile([128, SN * 3], F32, tag="f")
                    nc.sync.dma_start(
                        f[:],
                        src_slab_flat[ts(ci, CVOX * 3)]
                        .rearrange("(p x) -> p x", p=128))
                    f3 = f[:].rearrange("p (nv c) -> p nv c", c=3)

                    cg = []
                    for dc in range(3):
                        tu = sp.tile([128, SN], U8, tag=f"cu{dc}")
                        nc.sync.dma_start(
                            tu[:],
                            coordu[dc, :][ts(ci, CVOX)]
                            .rearrange("(p x) -> p x", p=128))
                        t = sp.tile([128, SN], F32, tag=f"cg{dc}")
                        nc.scalar.copy(out=t[:], in_=tu[:])
                        cg.append(t)
                    zg, yg, xg = cg          # integer voxel coords as f32

                    ws = []
                    for comp, (g, hi) in enumerate(((zg, D), (yg, H),
                                                    (xg, W))):
                        # p = clip(coord + flow*scale, 0, hi-1)
                        scale = (hi - 1) / 2.0
                        p_ = sp.tile([128, SN], F32, tag=f"p{comp}")
                        nc.vector.scalar_tensor_tensor(
                            out=p_[:], in0=f3[:, :, comp],
                            scalar=float(scale), in1=g[:],
                            op0=AL.mult, op1=AL.add)
                        nc.vector.tensor_scalar_max(p_[:], p_[:], 0.0)
                        nc.vector.tensor_scalar_min(p_[:], p_[:],
                                                    float(hi - 1))
                        # f32->i32 copy rounds to nearest; round(p-0.5) gives
                        # floor(p) (or floor-1 with weight 1 -- same lerp)
                        ph = sp.tile([128, SN], F32, tag=f"ph{comp}")
                        nc.vector.tensor_scalar_add(ph[:], p_[:], -0.5)
                        i0 = sp.tile([128, SN], I32, tag=f"i{comp}")
                        nc.vector.tensor_copy(out=i0[:], in_=ph[:])
                        f0 = sp.tile([128, SN], F32, tag=f"ff{comp}")
                        nc.vector.tensor_copy(out=f0[:], in_=i0[:])
                        nc.vector.tensor_scalar_min(f0[:], f0[:],
                                                    float(hi - 2))
                        w = sp.tile([128, SN], F32, tag=f"w{comp}")
                        nc.vector.tensor_tensor(out=w[:], in0=p_[:],
                                                in1=f0[:], op=AL.subtract)
                        ws.append((f0, w))
                    (z0f, wz), (y0f, wy), (x0f, wx) = ws

                    idxf = sp.tile([128, SN], F32, tag="idxf")
                    nc.vector.scalar_tensor_tensor(
                        out=idxf[:], in0=z0f[:], scalar=float(H), in1=y0f[:],
                        op0=AL.mult, op1=AL.add)
                    nc.vector.scalar_tensor_tensor(
                        out=idxf[:], in0=idxf[:], scalar=float(W), in1=x0f[:],
                        op0=AL.mult, op1=AL.add)
                    idxi = sp.tile([128, SN], I32, tag="idxi")
                    nc.vector.tensor_copy(out=idxi[:], in_=idxf[:])

                    g24 = sp.tile([128, SN * 24], F32, tag="g24")
                    g243 = g24[:].rearrange("p (nv k) -> p nv k", k=24)
                    if GATHER_MODE == "fused":
                        # HW emits one descriptor per 12-elem TAB row, one
                        # offset per run: interleave (idx, idx+1) so each
                        # voxel's 24-float window = rows idx, idx+1.
                        idx2 = sp.tile([128, SN * 2], I32, tag="idx2")
                        idx22 = idx2[:].rearrange("p (nv k) -> p nv k", k=2)
                        nc.vector.tensor_copy(out=idx22[:, :, 0],
                                              in_=idxi[:])
                        nc.vector.tensor_scalar_add(idx22[:, :, 1],
                                                    idxi[:], 1)
                        nc.gpsimd.indirect_dma_start(
                            out=g24[:], out_offset=None, in_=TAB,
                            in_offset=bass.IndirectOffsetOnAxis(
                                ap=idx2[:], axis=0))
                    elif GATHER_MODE == "perj":
                        for j in range(SN):
                            nc.gpsimd.indirect_dma_start(
                                out=g243[:, j, :], out_offset=None, in_=TAB,
                                in_offset=bass.IndirectOffsetOnAxis(
                                    ap=idxi[:, j:j + 1], axis=0))
                    else:  # plain: timing-structure stub, wrong results
                        nc.sync.dma_start(
                            g24[:],
                            TAB_flat[0:CVOX * 24]
                            .rearrange("(p x) -> p x", p=128))

                    # x-lerp (pairs 12 apart) -> z-lerp (6) -> y-lerp (3)
                    t1 = sp.tile([128, SN * 12], F32, tag="t1")
                    t13 = t1[:].rearrange("p (nv k) -> p nv k", k=12)
                    a, b = g243[:, :, 0:12], g243[:, :, 12:24]
                    nc.vector.tensor_tensor(out=t13, in0=b, in1=a,
                                            op=AL.subtract)
                    nc.vector.tensor_tensor(
                        out=t13, in0=t13,
                        in1=wx[:].to_broadcast([128, SN, 12]), op=AL.mult)
                    nc.vector.tensor_tensor(out=t13, in0=t13, in1=a,
                                            op=AL.add)
                    t2 = sp.tile([128, SN * 6], F32, tag="t2")
                    t23 = t2[:].rearrange("p (nv k) -> p nv k", k=6)
                    a, b = t13[:, :, 0:6], t13[:, :, 6:12]
                    nc.vector.tensor_tensor(out=t23, in0=b, in1=a,
                                            op=AL.subtract)
                    nc.vector.tensor_tensor(
                        out=t23, in0=t23,
                        in1=wz[:].to_broadcast([128, SN, 6]), op=AL.mult)
                    nc.vector.tensor_tensor(out=t23, in0=t23, in1=a,
                                            op=AL.add)
                    t3 = sp.tile([128, SN * 3], F32, tag="t3")
                    t33 = t3[:].rearrange("p (nv k) -> p nv k", k=3)
                    a, b = t23[:, :, 0:3], t23[:, :, 3:6]
                    nc.vector.tensor_tensor(out=t33, in0=b, in1=a,
                                            op=AL.subtract)
                    nc.vector.tensor_tensor(
                        out=t33, in0=t33,
                        in1=wy[:].to_broadcast([128, SN, 3]), op=AL.mult)
                    nc.vector.tensor_tensor(out=t33, in0=t33, in1=a,
                                            op=AL.add)

                    if not last:
                        outt = sp.tile([128, SN * 3], F32, tag="outt")
                        nc.vector.tensor_tensor(out=outt[:], in0=t3[:],
                                                in1=f[:], op=AL.add)
                        nc.sync.dma_start(
                            src_slab_flat[ts(ci, CVOX * 3)]
                            .rearrange("(p x) -> p x", p=128),
                            outt[:])
                    else:
                        # planar output: out_c = t33[:,:,c] + f3[:,:,c]
                        for c in range(3):
                            oc = sp.tile([128, SN], F32, tag=f"oc{c}")
                            nc.vector.tensor_tensor(
                                out=oc[:], in0=t33[:, :, c],
                                in1=f3[:, :, c], op=AL.add)
                            nc.sync.dma_start(
                                fout[c, :][ts(ci, CVOX)]
                                .rearrange("(p x) -> p x", p=128),
                                oc[:])
                with tc.For_i(0, NCH, 1) as ci:
                    sbody(ci)

            # ---- phase D: AllGather new slabs into FULL ----
            # (halves=2 split is rejected by the BIR verifier: collective
            # outs must be contiguous, and half-gathers stride across FULL)
            if not last:
                emit_allgather(f"ag{step}")

    nc.compile()
    return nc


def _get_nc():
    if "nc" not in _CACHE:
        _CACHE["nc"] = _build_nc()
    return _CACHE["nc"]


def _make_coords_u8():
    """Per-slab uint8 integer voxel coordinates, rows (z_global, y, x)."""
    if "coords" in _CACHE:
        return _CACHE["coords"]
    zz = np.arange(D, dtype=np.uint8)
    yy = np.arange(H, dtype=np.uint8)
    xx = np.arange(W, dtype=np.uint8)
    out = []
    for s in range(GROUP):
        z_v, y_v, x_v = np.meshgrid(zz[s * SLABZ:(s + 1) * SLABZ], yy, xx,
                                    indexing="ij")
        c = np.stack([z_v.ravel(), y_v.ravel(), x_v.ravel()])
        out.append(np.ascontiguousarray(c))
    _CACHE["coords"] = out
    return out


def kernel(flow: np.ndarray) -> np.ndarray:
    from concourse.bass_utils import run_bass_kernel_spmd

    assert flow.shape == (B, C, D, H, W), flow.shape
    nc = _get_nc()
    flow = np.ascontiguousarray(flow, dtype=np.float32)
    coords = _make_coords_u8()

    in_maps = []
    for core in range(NCORES):
        b, s = divmod(core, GROUP)
        zsl = slice(s * SLABZ, (s + 1) * SLABZ)
        m = {f"slab{c}": flow[b, c, zsl].reshape(-1) for c in range(3)}
        m["coordu"] = coords[s]
        in_maps.append(m)

    res = run_bass_kernel_spmd(nc, in_maps, core_ids=list(range(NCORES)))

    out = np.empty((B, C, D, H, W), dtype=np.float32)

    def _place(core):
        b, s = divmod(core, GROUP)
        zsl = slice(s * SLABZ, (s + 1) * SLABZ)
        out[b, :, zsl] = res.results[core]["fout"].reshape(3, SLABZ, H, W)

    from concurrent.futures import ThreadPoolExecutor
    with ThreadPoolExecutor(NCORES) as ex:
        list(ex.map(_place, range(NCORES)))
    return out
